# revision 29
# baseline (speedup 1.0000x reference)
"""ArDCA forward kernel for 8 trn2 NeuronCores.

z[m,i,a] = h[i,a] + sum_{j<i} sum_b J[i,j,b,a] * X[m,j,b]

Flattening (j,b)->K and (i,a)->N, this is one block-upper-triangular matmul
Z^T = Jmat^T @ X^T where J[i].reshape(L*Q, Q) is natively the i-th column
block of the stationary operand (no transpose of J needed).

Sharding: the 5376 output columns (i,a) are cut into 42 column-tiles of 128,
distributed over the 8 cores into 6 uniform slots per core (SPMD runs an
identical graph on every core; which column-tile a slot computes is decided
purely by the host-packed per-core J/h data — a slot whose tile needs fewer
K-tiles than the slot budget just gets zero-padded J). Each slot is one PSUM
accumulation chain: matmul(psum, lhsT=J_tile(128x128) bf16, rhs=XT_tile
(128x512) fp8) with f32 accumulation; a DVE tensor_scalar add of h evacuates
PSUM -> SBUF; the result is DMA'd out in f32. X^T (fp8: one-hot 0/1 is exact)
is resident in SBUF; J streams. All DRAM buffers are host-packed
partition-major so DMA descriptor runs per partition are >=512B.
"""

import math
import numpy as np
import ml_dtypes

M, L, Q = 512, 256, 21
LQ = L * Q                      # 5376 = 42*128
COLS = 128                      # output columns per group (column-tile)
NG = LQ // COLS                 # 42 column-tiles
NCORES = 8
NXT = LQ // 128                 # 42 X k-tiles
CKJ = 12                        # J k-tiles per DMA chunk
BF16 = ml_dtypes.bfloat16
FP8 = ml_dtypes.float8_e4m3


def _ktiles(g: int) -> int:
    i_max = (COLS * g + COLS - 1) // Q      # highest i in the tile
    return max(1, math.ceil(Q * i_max / 128))


def _plan():
    """Uniform slot structure + serpentine group->core assignment."""
    items = sorted(range(NG), key=lambda g: (-_ktiles(g), g))
    nslots = math.ceil(NG / NCORES)                      # 6
    budgets = [_ktiles(items[NCORES * r]) for r in range(nslots)]
    assign = [[None] * nslots for _ in range(NCORES)]    # assign[core][slot] = group
    for r in range(nslots):
        row = items[NCORES * r: NCORES * (r + 1)]
        for k, g in enumerate(row):
            core = k if r % 2 == 0 else NCORES - 1 - k
            assign[core][r] = g
    offs = [COLS * sum(budgets[:r]) for r in range(nslots)]  # jp col offset per slot
    return budgets, assign, offs


BUDGETS, ASSIGN, JOFFS = _plan()
S = len(BUDGETS)                 # 6 slots per core
WJ = COLS * sum(BUDGETS)         # jp total columns per core
WX = NXT * M                     # xt total columns (21504)
# ascending budgets: slot k first-touches only X tiles [B_{k-1}, B_k), so the
# X demand spreads over the whole run and a single consumption-order DMA
# stream (split round-robin over both HWDGE rings) can stay ahead of the PE
SLOT_ORDER = sorted(range(S), key=lambda r: BUDGETS[r])
# X chunk k = the tiles slot k first-touches
XCHUNKS = tuple(
    BUDGETS[SLOT_ORDER[k]] - (BUDGETS[SLOT_ORDER[k - 1]] if k else 0)
    for k in range(S)
)


def _build_nc():
    import concourse.bacc as bacc
    import concourse.mybir as mybir
    from concourse import tile

    f32 = mybir.dt.float32
    bf16 = mybir.dt.bfloat16
    fp8 = mybir.dt.float8e4

    nc = bacc.Bacc(None, target_bir_lowering=False, debug=False)
    xt_ext = nc.declare_dram_parameter("xt", [128, WX], fp8, isOutput=False)
    jp_ext = nc.declare_dram_parameter("jp", [128, WJ], bf16, isOutput=False)
    hb_ext = nc.declare_dram_parameter("hb", [COLS, S], f32, isOutput=False)
    out_ext = nc.declare_dram_parameter("out", [S * COLS, M], f32, isOutput=True)

    with tile.TileContext(nc) as tc:
        with (
            tc.tile_pool(name="x", bufs=1) as xpool,
            tc.tile_pool(name="j", bufs=1) as jpool,
            tc.tile_pool(name="ps", bufs=6, space="PSUM") as ppool,
            tc.tile_pool(name="o", bufs=6) as opool,
            tc.tile_pool(name="c", bufs=1) as cpool,
        ):

            # HAM warm-up: the PE clock-gate releases (1.2 -> 2.4 GHz) only
            # after ~3.4us of sustained matmul activity. Run zero-weight
            # matmuls during the DMA preamble dead-time, folded into slot 0's
            # accumulation chain (they add 0 to the psum, so they are neither
            # dead code nor a numerics change). The memset must be gpsimd's
            # first op so the PE can start warming as early as possible.
            NWARM = 9
            zw = cpool.tile([128, 128], bf16, tag="zw")
            nc.vector.memset(zw[:], 0.0)
            # rhs for the dummies: the same 128 zero columns read 4x via a
            # zero-stride AP dim -> free size 512 with only a 32KB memset
            import concourse.bass as _bass
            _a = zw[:]
            zw_rhs = _bass.AP(_a.tensor, _a.offset,
                              [_a.ap[0], (0, M // 128), _a.ap[1]])
            hb_t = cpool.tile([COLS, S], f32, tag="hb")
            nc.gpsimd.dma_start(out=hb_t[:], in_=hb_ext[:])

            # one global DMA stream in exact consumption order, split over the
            # two HWDGE rings greedily by queued bytes (each ring is FIFO, so
            # balanced byte loads keep arrival order ~= consumption order);
            # every tile unique-tagged and resident (no pool-recycle waits)
            rings = [nc.sync, nc.scalar]
            ring_bytes = [0, 0]

            def ring_dma(out_ap, in_ap, nbytes):
                i = 0 if ring_bytes[0] <= ring_bytes[1] else 1
                rings[i].dma_start(out=out_ap, in_=in_ap)
                ring_bytes[i] += nbytes

            xts = []           # (tile, local_ktile) per global X ktile
            jtiles = {}        # (slot, chunk_start) -> tile
            xoff = 0

            def emit_x(si, cx):
                nonlocal xoff
                # split a slot's fresh X window into <=4-tile items so arrival
                # is incremental
                done = 0
                while done < cx:
                    n = min(4, cx - done)
                    xt_t = xpool.tile([128, n * M], fp8, tag=f"x{si}_{done}")
                    ring_dma(xt_t[:], xt_ext[:, xoff * M:(xoff + n) * M],
                             n * M * 128)
                    for t in range(n):
                        xts.append((xt_t, t))
                    xoff += n
                    done += n

            def jchunks(T):
                cs, t = [], 0
                while t < T:
                    ck = min(CKJ, T - t)
                    cs.append((t, ck))
                    t += ck
                return cs

            for si, r in enumerate(SLOT_ORDER):
                T = BUDGETS[r]
                cs = jchunks(T)
                # slot 0 touches its X tiles immediately; later slots touch
                # their fresh X window only from chain position B_{k-1}, i.e.
                # after their first J chunk is already being consumed
                if si == 0:
                    emit_x(si, XCHUNKS[si])
                for idx, (t, ck) in enumerate(cs):
                    jt = jpool.tile([128, ck * COLS], bf16, tag=f"j{r}_{t}")
                    c0 = JOFFS[r] + t * COLS
                    ring_dma(jt[:], jp_ext[:, c0:c0 + ck * COLS], ck * COLS * 256)
                    jtiles[(r, t)] = jt
                    if si > 0 and idx == 0:
                        emit_x(si, XCHUNKS[si])


            for si, r in enumerate(SLOT_ORDER):
                T = BUDGETS[r]
                ps = ppool.tile([COLS, M], f32, tag="ps")
                if si == 0:
                    for w in range(NWARM):
                        nc.tensor.matmul(
                            ps[:], zw[:], zw_rhs,
                            start=(w == 0), stop=False,
                        )
                for t, ck in jchunks(T):
                    jt = jtiles[(r, t)]
                    for tl in range(ck):
                        tt = t + tl
                        xt_t, xl = xts[tt]
                        nc.tensor.matmul(
                            ps[:],
                            jt[:, tl * COLS:(tl + 1) * COLS],
                            xt_t[:, xl * M:(xl + 1) * M],
                            start=(tt == 0 and si != 0),
                            stop=(tt == T - 1),
                        )
                ot = opool.tile([COLS, M], f32, tag="ot")
                if si == S - 1:
                    # final slot: split evac+store in halves across both
                    # now-idle HWDGE rings to shorten the completion tail
                    H = M // 2
                    nc.vector.tensor_scalar_add(
                        ot[:, :H], ps[:, :H], hb_t[:, r:r + 1])
                    nc.sync.dma_start(
                        out=out_ext[r * COLS:(r + 1) * COLS, :H], in_=ot[:, :H])
                    nc.vector.tensor_scalar_add(
                        ot[:, H:], ps[:, H:], hb_t[:, r:r + 1])
                    nc.scalar.dma_start(
                        out=out_ext[r * COLS:(r + 1) * COLS, H:], in_=ot[:, H:])
                else:
                    # stores on SWDGE so they never delay the HWDGE load rings
                    nc.vector.tensor_scalar_add(ot[:], ps[:], hb_t[:, r:r + 1])
                    nc.gpsimd.dma_start(
                        out=out_ext[r * COLS:(r + 1) * COLS, :], in_=ot[:])

    nc.finalize()
    return nc


_CACHE = {}


def _get_nc():
    if "nc" not in _CACHE:
        _CACHE["nc"] = _build_nc()
    return _CACHE["nc"]


def _pack_inputs(X_oh, h_pos, J):
    """Build per-core in_maps (host-side shard + layout)."""
    XT = np.ascontiguousarray(X_oh.transpose(1, 2, 0).reshape(LQ, M))
    xt = np.ascontiguousarray(
        XT.reshape(NXT, 128, M).transpose(1, 0, 2).reshape(128, WX)
    ).astype(FP8)

    JT = J.reshape(L, LQ, Q).astype(BF16)   # JT[i] = (jb, a) column block of i
    h32 = h_pos.astype(np.float32)

    in_maps = []
    for core in range(NCORES):
        jp = np.zeros((128, WJ), dtype=BF16)
        hb = np.zeros((COLS, S), dtype=np.float32)
        for r in range(S):
            g = ASSIGN[core][r]
            if g is None:
                continue
            T = BUDGETS[r]
            blk = np.zeros((T * 128, COLS), dtype=BF16)
            # columns are global output indices ia = COLS*g + col, i = ia//Q
            ia0 = COLS * g
            col = 0
            while col < COLS:
                i, a0 = divmod(ia0 + col, Q)
                na = min(Q - a0, COLS - col)        # run of columns within one i
                rows = Q * i                        # strictly-lower mask: j < i
                blk[:rows, col:col + na] = JT[i][:rows, a0:a0 + na]
                hb[col:col + na, r] = h32[i, a0:a0 + na]
                col += na
            jp[:, JOFFS[r]:JOFFS[r] + T * COLS] = (
                blk.reshape(T, 128, COLS).transpose(1, 0, 2).reshape(128, T * COLS)
            )
        in_maps.append({"xt": xt, "jp": jp, "hb": hb})
    return in_maps


def _unpack_outputs(results):
    outT = np.zeros((LQ, M), dtype=np.float32)
    for core in range(NCORES):
        o = results[core]["out"]
        for r in range(S):
            g = ASSIGN[core][r]
            if g is None:
                continue
            outT[COLS * g:COLS * (g + 1)] = o[r * COLS:(r + 1) * COLS]
    return np.ascontiguousarray(outT.reshape(L, Q, M).transpose(2, 0, 1))


def _run(in_maps, trace=False, **kw):
    from concourse.bass_utils import run_bass_kernel_spmd

    nc = _get_nc()
    return run_bass_kernel_spmd(nc, in_maps, list(range(NCORES)), trace=trace, **kw)


def kernel(X_oh, h_pos, J):
    X_oh = np.asarray(X_oh, dtype=np.float32)
    h_pos = np.asarray(h_pos, dtype=np.float32)
    J = np.asarray(J, dtype=np.float32)
    in_maps = _pack_inputs(X_oh, h_pos, J)
    res = _run(in_maps)
    return _unpack_outputs(res.results)
